# revision 23
# baseline (speedup 1.0000x reference)
"""Trainium2 Bass kernel v10: multi-head self-attention block.

v9 -> v10: the PE was the wall (87% busy, throttled to ~1.2GHz effective by
power management).  attn@v was its biggest stream (131K columns in bf16), so
v10 moves it to fp8-e4m3 DoubleRow (0.5 cyc/col, 256-row contraction),
halving it to 65K cycles, without giving up precision via two tricks:

 1. expm1 pullback: the PE streams f = exp(s) - 1 (computed by a DVE
    tensor_scalar pass over the ACT exp output, SBUF->SBUF at the 2x_2p
    rate) instead of exp(s).  Scores are small (std ~0.25), so |f| ~ 0.28
    and e4m3's 3.6% relative error applies to f, not to exp ~ 1.0: the
    attention-level quantization error drops ~4x vs quantizing exp
    directly.  The dropped "+1" is restored exactly by adding
    colsum_h(v) = sum_j v[j,:] -- host-precomputed from the same bf16
    inputs the device sees -- to the psum result, and by a +2048 offset on
    the softmax denominator Z (the ones-columns of the stationary then
    accumulate sum_j f_j, and sum_j 1 = 2048 exactly).
 2. v residual pair: the stationary holds v8 = e4m3(v) and, in a second
    accumulated matmul, vr8 = e4m3(v - v8) (unscaled; e4m3 subnormals
    bound the absolute error at ~1e-3).  Both accumulate into the same
    psum group, so v's effective precision is ~11 bits at fp8 stream cost.

Per-core PE stream cycles: qkv proj 98K (bf16) + scores 65K (fp8 DR) +
attn@v 65K (fp8 DR, was 131K) + out proj 33K (bf16) = ~265K (was 330K).
Measured: 314us -> 251us (rel err 1.70e-2, budget 2e-2); the
out-projection fillers start right after their gating normalize (steps
26/37/49), spreading PE work into ACT-gated mid-stream slack.

The softmax denominator reciprocal moved to reciprocal_approx_fast (18
bits, 5x faster) to make room on DVE for the f8 subtract stream.  NOTE
custom-DVE ops require in/out to start on the SAME partition (offset
mismatch = NaN at runtime), and the BIR verifier requires tensor_tensor's
two INPUTS to share a start partition -- hence the azo/azz split tiles.

The front x DMAs ride the scalar engine's HWDGE queue so they overlap
the w pieces on sync (~15us off the start ramp).

Tried and REVERTED (slower on HW):
  - fp8-DR residual-triplet qkv projections (x8@w8+x8@wr8+xr8@w8): -37K
    stream cycles but +150 LDWEIGHTS on short (128-256 cyc) streams;
    walrus emits a weight load per matmul with no reuse/elision, so the
    loads dominate and PE busy went +58us.  (Numerically it was fine at
    1.5e-2 WITH operand pre-scaling: w_q/w_k/w_v std 1/1024..1/32 sit in
    e4m3's subnormal range, so scale to O(1) and undo via the exp input
    scale (2^-15) and a host-side wo fold.)
  - merging the q fillers into 512-token units: 4.6K-cycle monolithic
    blocks exceed the ~2us per-step filler slack and stall the exp
    stream (+70us).  Filler granularity must stay under the step budget.

DoubleRow layout for scores: head-dim 64 split lo/hi (d<32 / d>=32);
q8/k8 are [128, 2, n] fp8 tiles with q8[32h+p, i, n] = q_h[d=32i+p, n].
The host pre-permutes w_qkv's q/k column blocks [h0.lo|h1.lo|...|h0.hi|...]
so the projection psum lands directly in this layout.  For attn@v the
contraction is j: f8 tiles [128, 2, 512] with f8[p, jt2, i] =
f[j=256*jb+128*jt2+p, i] are already in DR layout; voA/voB stationaries
[p, jb, r, h, slot, d] hold v8/ones and vr8/zeros with j = 256*jb+128*r+p.

Structure (program order = dataflow order; consumers after producers).
  Phase A: k (all 4 chunks, chasing the per-chunk x DMAs) + q(chunk 0)
    only -- the minimum that gates the first score matmul.
  Phase B: one flat software-pipelined stream over the 64 (ic, pr,
    j-block) steps: scores(n) -> exp(n) [ACT] -> f8(n) [DVE] ->
    attn@v(n-LAG) [PE, DR].  attn@v runs LAG=10 steps behind while the v
    projections drip in as the first PE fillers, then the lag decays to 1.
    q(c1..c3) in token-half units and out-proj(ic) right after each
    (ic, pr=1) normalize fill the remaining PE slack.
  Tail: out-projection of the last chunk only, its PSUM tiles cycled
    through the idle scores/avz banks, copies split ACT/DVE.

Engines: PE matmuls; ACT exp (plus tail copies); DVE f8 subtract stream,
PSUM->SBUF copies (fused with the colsum/+2048 offsets via a per-partition
scalar AP), softmax normalization; GpSimd memsets; SP DMAs.
attn@v keeps the [v | ones] stationary trick: psum rows 0-63 hold the
unnormalized output minus colsum, rows 64-127 the denominator Z - 2048
replicated; normalization is reciprocal + elementwise multiply on DVE.
"""

import numpy as np
import ml_dtypes

import concourse.bacc as bacc
import concourse.mybir as mybir
import concourse.tile as tile
from concourse.bass_utils import run_bass_kernel_spmd

P = 128
DIM = 1024
HEADS = 16
B = 2
N = 2048
NCORES = 8
HGROUPS = 4                     # head-groups (tensor parallel)
H_LOC = HEADS // HGROUPS        # 4 heads per core
DH = DIM // HEADS               # 64
F_LOC = H_LOC * DH              # 256 features per core (per q/k/v)
SCALE = DIM ** -0.5             # exactly 1/32

F32 = mybir.dt.float32
BF16 = mybir.dt.bfloat16
F8 = mybir.dt.float8e4
EXP = mybir.ActivationFunctionType.Exp
IDENT = mybir.ActivationFunctionType.Identity
ADD = mybir.AluOpType.add
SUB = mybir.AluOpType.subtract
DR = mybir.MatmulPerfMode.DoubleRow

IC = N // 512                   # query chunks of 512
JT = N // P                     # key tiles of 128
NJB = JT // 2                   # j-blocks of 2 key tiles


def build_nc(kt: int):
    """Single-core program (identical on all 8 cores).

    kt: number of 128-row contraction tiles for the qkv projection
        (8 for dim=1024, 9 when a ones-row block is appended for biases).
    """
    nc = bacc.Bacc(trn_type="TRN2")

    xT = nc.dram_tensor("xT", (kt * P, N), BF16, kind="ExternalInput")
    w = nc.dram_tensor("w", (kt * P, 3 * F_LOC), BF16, kind="ExternalInput")
    wo = nc.dram_tensor("wo", (F_LOC, DIM), BF16, kind="ExternalInput")
    csum = nc.dram_tensor("csum", (P, H_LOC + 1), F32, kind="ExternalInput")
    out = nc.dram_tensor("out", (N, DIM), BF16, kind="ExternalOutput")

    xT_t = xT[:].rearrange("(t p) n -> p t n", p=P)        # [128, kt, N]
    w_t = w[:].rearrange("(t p) f -> p t f", p=P)          # [128, kt, 768]
    wo_t = wo[:].rearrange("(t p) e -> p t e", p=P)        # [128, 2, 1024]

    with tile.TileContext(nc) as tc:
        with (
            tc.tile_pool(name="persist", bufs=1) as persist,
        ):
            w_sb = persist.tile([P, kt, 3 * F_LOC], BF16, tag="w")
            xt = persist.tile([P, kt, N], BF16, tag="xt")  # all 4 chunks
            wo_sb = persist.tile([P, 2, DIM], BF16, tag="wo")
            # per-head [colsum | 2048] column for the f8 pullback offsets:
            # rows 0-63 = sum_j v[j, 64h+d], rows 64-127 = 2048.0
            zc = persist.tile([P, H_LOC + 1], F32, tag="zc")
            # fp8 q/k in DoubleRow layout: [32h+p, i, n] = q_h[d=32i+p, n]
            q8 = persist.tile([P, 2, N], F8, tag="q8")
            k8 = persist.tile([P, 2, N], F8, tag="k8")
            # attn@v DR stationaries, j = 256*jb + 128*r + p:
            #   voA[p, jb, r, h, 0, d] = e4m3(v), [.., 1, d] = 1.0
            #   voB[p, jb, r, h, 0, d] = e4m3(v - voA), [.., 1, d] = 0.0
            voA = persist.tile([P, NJB, 2, H_LOC, 2, DH], F8, tag="voA")
            voB = persist.tile([P, NJB, 2, H_LOC, 2, DH], F8, tag="voB")
            outT = persist.tile([P, 2, N], BF16, tag="outT")  # [hd, kp, tok]

            scratch = persist.tile([P, 512], BF16, tag="scratch")
            nc.gpsimd.memset(scratch, 1.0)
            nc.gpsimd.memset(voA[:, :, :, :, 1, :], 1.0)
            nc.gpsimd.memset(voB[:, :, :, :, 1, :], 0.0)
            # Front DMAs ordered along the critical chain: phase A consumes
            # x chunk-by-chunk against the q/k then v columns of w.  Each
            # dma_start costs ~625ns serial on HWDGE, so pieces are sized to
            # keep the PE chasing the transfers without DMA-count bloat.
            nc.sync.dma_start(out=w_sb[:, 0:1, 0:512], in_=w_t[:, 0:1, 0:512])
            nc.scalar.dma_start(out=xt[:, 0:1, 0:512],
                                in_=xT_t[:, 0:1, 0:512])
            nc.sync.dma_start(out=w_sb[:, 1:2, 0:512], in_=w_t[:, 1:2, 0:512])
            nc.scalar.dma_start(out=xt[:, 1:2, 0:512],
                                in_=xT_t[:, 1:2, 0:512])
            nc.sync.dma_start(out=w_sb[:, 2:kt, 0:512], in_=w_t[:, 2:kt, 0:512])
            nc.scalar.dma_start(out=xt[:, 2:kt, 0:512],
                                in_=xT_t[:, 2:kt, 0:512])
            nc.scalar.dma_start(out=xt[:, :, 512:1024],
                                in_=xT_t[:, :, 512:1024])
            nc.scalar.dma_start(out=xt[:, :, 1024:1536],
                                in_=xT_t[:, :, 1024:1536])
            nc.scalar.dma_start(out=xt[:, :, 1536:2048],
                                in_=xT_t[:, :, 1536:2048])
            nc.sync.dma_start(out=w_sb[:, :, 512:768], in_=w_t[:, :, 512:768])
            nc.sync.dma_start(out=wo_sb, in_=wo_t)
            nc.sync.dma_start(out=zc, in_=csum[:])

            # ---- Phase A: k (all chunks) + q chunk 0 -------------------
            # v projections drip in as stream fillers; attn@v runs a few
            # steps behind the exp stream until they land.
            with (
                tc.tile_pool(name="ps_kq", bufs=4, space="PSUM") as ps_kq,
            ):
                def proj_kq_a(which, dst, half, c):
                    f0 = which * F_LOC + half * P
                    csl = slice(c * 512, (c + 1) * 512)
                    ps = ps_kq.tile([P, 512], F32, tag="pskq",
                                    name=f"pA{which}_{half}_{c}")
                    for k in range(kt):
                        nc.tensor.matmul(
                            ps,
                            lhsT=w_sb[:, k, f0:f0 + P],
                            rhs=xt[:, k, csl],
                            start=(k == 0),
                            stop=(k == kt - 1),
                        )
                    nc.vector.tensor_copy(dst[:, half, csl], ps)

                # warmup: keep the PE busy during the initial DMA latency
                # so the p-state ramp (full clock after 3us of continuous
                # busy) is underway before the real projections start
                wps = ps_kq.tile([P, 512], F32, tag="pskq", name="warm_ps")
                for r in range(6):
                    nc.tensor.matmul(
                        wps,
                        lhsT=scratch[:, 0:P],
                        rhs=scratch,
                        start=(r == 0),
                        stop=(r == 5),
                    )
                for half in range(2):
                    proj_kq_a(1, k8, half, 0)              # k chunk 0
                for half in range(2):                      # q chunk 0
                    proj_kq_a(0, q8, half, 0)

            # ---- Phase B: flat attention pipeline ----------------------
            with (
                tc.tile_pool(name="ps_s", bufs=2, space="PSUM") as ps_s,
                tc.tile_pool(name="ps_avz0", bufs=1, space="PSUM") as ps_avz0,
                tc.tile_pool(name="ps_avz1", bufs=1, space="PSUM") as ps_avz1,
                tc.tile_pool(name="ps_aux", bufs=2, space="PSUM") as ps_aux,
                tc.tile_pool(name="expp", bufs=6) as exp_pool,
                tc.tile_pool(name="f8p", bufs=24) as f8_pool,
                tc.tile_pool(name="azp", bufs=8) as az_pool,
                tc.tile_pool(name="rzp", bufs=6) as rz_pool,
                tc.tile_pool(name="osb", bufs=16) as out_pool,
            ):
                def q_filler(c, half, th):
                    tsl = slice(c * 512 + th * 256, c * 512 + (th + 1) * 256)
                    f0 = half * P
                    ps = ps_aux.tile([P, 256], F32, tag="aux",
                                     name=f"q_{c}_{half}_{th}")
                    for k in range(kt):
                        nc.tensor.matmul(
                            ps,
                            lhsT=w_sb[:, k, f0:f0 + P],
                            rhs=xt[:, k, tsl],
                            start=(k == 0),
                            stop=(k == kt - 1),
                        )
                    nc.vector.tensor_copy(q8[:, half, tsl], ps)

                def p3_step(c, st, tail=False):
                    # out-projection: token tile it of chunk c, half ec
                    it, ec = c * 4 + st // 2, st % 2
                    i0 = it * P
                    esl = slice(ec * 512, (ec + 1) * 512)
                    if tail:
                        # scores/avz psum is idle in the tail: cycle po
                        # tiles through those slots so the mm->copy->release
                        # convoy runs 4+ wide instead of 2
                        pool, tg = [(ps_s, "s"), (ps_avz0, "avz0"),
                                    (ps_avz1, "avz1"), (ps_aux, "aux")][st % 4]
                    else:
                        pool, tg = ps_aux, "aux"
                    po = pool.tile([P, 512], F32, tag=tg,
                                   name=f"po_{it}_{ec}")
                    for kp in range(2):
                        nc.tensor.matmul(
                            po,
                            lhsT=outT[:, kp, i0:i0 + P],
                            rhs=wo_sb[:, kp, esl],
                            start=(kp == 0),
                            stop=(kp == 1),
                        )
                    po_sb = out_pool.tile([P, 512], BF16, tag="po_sb",
                                          name=f"posb_{it}_{ec}")
                    if tail and st % 2 == 0:
                        nc.scalar.copy(out=po_sb, in_=po)  # ACT idle in tail
                    else:
                        nc.vector.tensor_copy(po_sb, po)
                    if tail and st % 2 == 1:
                        nc.scalar.dma_start(out=out[i0:i0 + P, esl],
                                            in_=po_sb)
                    else:
                        nc.sync.dma_start(out=out[i0:i0 + P, esl], in_=po_sb)

                def k_filler(c, half):
                    csl = slice(c * 512, (c + 1) * 512)
                    f0 = F_LOC + half * P
                    ps = ps_aux.tile([P, 512], F32, tag="aux",
                                     name=f"kB_{c}_{half}")
                    for k in range(kt):
                        nc.tensor.matmul(
                            ps,
                            lhsT=w_sb[:, k, f0:f0 + P],
                            rhs=xt[:, k, csl],
                            start=(k == 0),
                            stop=(k == kt - 1),
                        )
                    nc.vector.tensor_copy(k8[:, half, csl], ps)

                def proj_v(c, tt):
                    t0 = c * 512 + tt * P
                    jb, r = 2 * c + tt // 2, tt % 2
                    psv = ps_aux.tile([P, F_LOC], F32, tag="aux",
                                      name=f"v_{c}_{tt}")
                    for k in range(kt):
                        nc.tensor.matmul(
                            psv,
                            lhsT=xt[:, k, t0:t0 + P],
                            rhs=w_sb[:, k, 2 * F_LOC:3 * F_LOC],
                            start=(k == 0),
                            stop=(k == kt - 1),
                        )
                    # v8 then the e4m3 residual (v - v8); both accumulate in
                    # the same psum group at attn@v time
                    nc.vector.tensor_copy(voA[:, jb, r, :, 0, :], psv)
                    nc.vector.tensor_tensor(
                        out=voB[:, jb, r, :, 0, :],
                        in0=psv,
                        in1=voA[:, jb, r, :, 0, :],
                        op=SUB,
                    )

                # Per-step filler schedule. attn@v runs D steps behind the
                # exp stream (D: 10 -> 1 as v projections land), so v(c) at
                # steps 2c..2c+1 is always emitted before its attn@v
                # consumer; q(c) lands before scores(ic=c); p3(c) after
                # normalize(c, pr=1).
                import collections
                step_fillers = collections.defaultdict(list)
                for c in (1, 2, 3):                    # k(c) by step 2c
                    for half in range(2):
                        step_fillers[2 * (c - 1) + half].append(
                            lambda c=c, half=half: k_filler(c, half))
                v_units = [(c, tt) for c in range(IC) for tt in range(4)]
                pat = [2, 2, 2, 2, 1, 1, 1, 1, 1, 1, 1, 1]  # 16 units
                i = 0
                for s, k_ in enumerate(pat):
                    for _ in range(k_):
                        c, tt = v_units[i]
                        step_fillers[6 + s].append(
                            lambda c=c, tt=tt: proj_v(c, tt))
                        i += 1
                # q(c1) must be fully emitted before step 16 reads it
                qsched = {1: [12, 13, 14, 15], 2: [18, 19, 20, 21],
                          3: [22, 23, 24, 25]}
                for c in range(1, IC):
                    for u, s in enumerate(qsched[c]):
                        half, th = u // 2, u % 2
                        step_fillers[s].append(
                            lambda c=c, half=half, th=th:
                            q_filler(c, half, th))
                # p3(c) must follow normalize(c, pr=1): flush(15) lands at
                # step 25, flush(31) at 36, flush(47) at 48 under lag(n) --
                # start each chunk's out-projection right after, spreading
                # the PE work into the ACT-gated mid-stream slack
                p3base = {0: 26, 1: 37, 2: 49}
                for c in range(IC - 1):
                    for st in range(8):
                        step_fillers[p3base[c] + st].append(
                            lambda c=c, st=st: p3_step(c, st))

                TOT = IC * 2 * NJB
                avzp = (ps_avz0, ps_avz1)
                blocks = {}

                def decode(n):
                    icpr, jb = divmod(n, NJB)
                    ic, pr = divmod(icpr, 2)
                    return ic, pr, jb

                def flush_avz(n, f8s):
                    # DR attn@v: contraction 256 j per matmul (the step's
                    # whole f8 tile); A = [v8 | ones], B = [vr8 | zeros]
                    # accumulate into one psum group per (ic, pr, h2)
                    ic, pr, jb = decode(n)
                    avz = blocks[(ic, pr)]
                    if n == TOT - 1:
                        # token-split to chase the split last-step exp;
                        # cols 0:255 of the group complete after half 0, so
                        # the (slice-level-dependent) normalize quarters
                        # start while half 1 is still streaming
                        for half in range(2):
                            hsl = slice(half * 256, (half + 1) * 256)
                            for h2 in range(2):
                                h = pr * 2 + h2
                                nc.tensor.matmul(
                                    avz[h2][:, hsl],
                                    lhsT=voA[:, jb, :, h, :, :],
                                    rhs=f8s[h2][:, :, hsl],
                                    start=False,
                                    stop=False,
                                    perf_mode=DR,
                                    skip_group_check=True,
                                )
                                nc.tensor.matmul(
                                    avz[h2][:, hsl],
                                    lhsT=voB[:, jb, :, h, :, :],
                                    rhs=f8s[h2][:, :, hsl],
                                    start=False,
                                    stop=(half == 1),
                                    perf_mode=DR,
                                    skip_group_check=True,
                                )
                        return
                    for h2 in range(2):
                        h = pr * 2 + h2
                        first = jb == 0
                        last = jb == NJB - 1
                        nc.tensor.matmul(
                            avz[h2],
                            lhsT=voA[:, jb, :, h, :, :],
                            rhs=f8s[h2],
                            start=first,
                            stop=False,
                            perf_mode=DR,
                            skip_group_check=True,
                        )
                        nc.tensor.matmul(
                            avz[h2],
                            lhsT=voB[:, jb, :, h, :, :],
                            rhs=f8s[h2],
                            start=False,
                            stop=last,
                            perf_mode=DR,
                            skip_group_check=True,
                        )

                def normalize(ic, pr):
                    # psum rows 0-63 hold attn@v minus colsum, rows 64-127
                    # hold Z-2048.  Split the PSUM->SBUF move into two
                    # 64-partition tiles, both landing at partition 0: the
                    # custom-DVE reciprocal needs matching in/out start
                    # partitions, and tensor_tensor needs both INPUTS to
                    # share a start partition.  The colsum offset rides the
                    # out-copy as a per-partition scalar AP; the Z offset
                    # (+2048) rides the Z-copy as an immediate.  For the
                    # very last block ACT is idle, so its copies go there to
                    # shorten the serial DVE chain ahead of the final
                    # out-projection.
                    isl = slice(ic * 512, (ic + 1) * 512)
                    avz = blocks.pop((ic, pr))
                    last = ic == IC - 1 and pr == 1
                    parts = []
                    for h2 in range(2):
                        h = pr * 2 + h2
                        azo = az_pool.tile([DH, 512], F32, tag="azo",
                                           name=f"azo{h2}_{ic}_{pr}")
                        azz = az_pool.tile([DH, 512], F32, tag="azz",
                                           name=f"azz{h2}_{ic}_{pr}")
                        if last and h2 == 1:
                            # ACT takes head h2=1's copies (both engines
                            # idle here) so the first quarter-normalize
                            # starts as early as possible
                            nc.scalar.activation(
                                azo, avz[h2][0:DH, :], IDENT,
                                bias=zc[0:DH, h:h + 1])
                            nc.scalar.activation(
                                azz, avz[h2][DH:P, :], IDENT,
                                bias=zc[0:DH, H_LOC:H_LOC + 1])
                        else:
                            nc.vector.tensor_scalar(
                                out=azo, in0=avz[h2][0:DH, :],
                                scalar1=zc[0:DH, h:h + 1], scalar2=None,
                                op0=ADD)
                            nc.vector.tensor_scalar(
                                out=azz, in0=avz[h2][DH:P, :],
                                scalar1=2048.0, scalar2=None,
                                op0=ADD)
                        parts.append((azo, azz))
                    if last:
                        # split the final block's normalize into 128-token
                        # quarters so the tail out-projection's first tiles
                        # start earlier (its units consume 128-token
                        # slices of outT)
                        for th in range(4):
                            tsl = slice(ic * 512 + th * P,
                                        ic * 512 + (th + 1) * P)
                            psl = slice(th * P, (th + 1) * P)
                            for h2 in range(2):
                                azo, azz = parts[h2]
                                osl = slice(h2 * DH, (h2 + 1) * DH)
                                rz = rz_pool.tile([DH, P], F32, tag="rz")
                                nc.vector.reciprocal_approx_fast(
                                    out=rz, in_=azz[:, psl])
                                nc.vector.tensor_mul(
                                    out=outT[osl, pr, tsl],
                                    in0=azo[:, psl],
                                    in1=rz,
                                )
                    else:
                        for h2 in range(2):
                            azo, azz = parts[h2]
                            osl = slice(h2 * DH, (h2 + 1) * DH)
                            rz = rz_pool.tile([DH, 512], F32, tag="rz")
                            nc.vector.reciprocal_approx_fast(
                                out=rz, in_=azz)
                            nc.vector.tensor_mul(
                                out=outT[osl, pr, isl],
                                in0=azo,
                                in1=rz,
                            )

                def lag(n):
                    # attn@v pipeline depth: 10 while k1-3/v project,
                    # decaying to the minimum 1 by step ~46
                    return 10 if n < 28 else max(1, 10 - (n - 26) // 2)

                pend_avs = collections.deque()   # (step m, f8 tiles)
                for n in range(TOT + 1):
                    if n < TOT:
                        ic, pr, jb = decode(n)
                        isl = slice(ic * 512, (ic + 1) * 512)
                        if jb == 0:
                            blocks[(ic, pr)] = [
                                avzp[h2].tile([P, 512], F32, tag=f"avz{h2}",
                                              name=f"avz{h2}_{ic}_{pr}")
                                for h2 in range(2)
                            ]
                        ss = [ps_s.tile([P, 2, 512], F32, tag="s",
                                        name=f"s{ic}_{pr}_{jb}_{h2}")
                              for h2 in range(2)]
                        for jt2 in range(2):
                            jt = jb * 2 + jt2
                            for h2 in range(2):
                                h = pr * 2 + h2
                                hsl = slice(32 * h, 32 * h + 32)
                                nc.tensor.matmul(
                                    ss[h2][:, jt2, :],
                                    lhsT=k8[hsl, :, jt * P:(jt + 1) * P],
                                    rhs=q8[hsl, :, isl],
                                    start=True,
                                    stop=True,
                                    perf_mode=DR,
                                    skip_group_check=True,
                                    tile_position=(32 * h, 0),
                                )
                        f8s = []
                        if n == TOT - 1:
                            # split the last step along tokens so the final
                            # flush/normalize/out-proj chain starts after
                            # the first half-exp instead of the whole step
                            es = [exp_pool.tile([P, 2, 512], BF16,
                                                tag="exp", name=f"eL{h2}")
                                  for h2 in range(2)]
                            f8s = [f8_pool.tile([P, 2, 512], F8, tag="f8",
                                                name=f"fL{h2}")
                                   for h2 in range(2)]
                            for half in range(2):
                                hsl = slice(half * 256, (half + 1) * 256)
                                for h2 in range(2):
                                    nc.scalar.activation(
                                        es[h2][:, :, hsl],
                                        ss[h2][:, :, hsl], EXP)
                                    nc.vector.tensor_scalar(
                                        out=f8s[h2][:, :, hsl],
                                        in0=es[h2][:, :, hsl],
                                        scalar1=-1.0, scalar2=None, op0=ADD)
                        else:
                            for h2 in range(2):
                                e = exp_pool.tile([P, 2, 512], BF16,
                                                  tag="exp",
                                                  name=f"e{ic}_{pr}_{jb}_{h2}")
                                nc.scalar.activation(e, ss[h2], EXP)
                                f = f8_pool.tile([P, 2, 512], F8, tag="f8",
                                                 name=f"f{ic}_{pr}_{jb}_{h2}")
                                nc.vector.tensor_scalar(
                                    out=f, in0=e, scalar1=-1.0, scalar2=None,
                                    op0=ADD)
                                f8s.append(f)
                        pend_avs.append((n, f8s))
                    # fillers first: their writes (v, q, outT consumers)
                    # must be emitted before the attn@v / scores that read
                    for f in step_fillers.get(n, []):
                        f()
                    while pend_avs and n - pend_avs[0][0] >= lag(n):
                        m, f8s_m = pend_avs.popleft()
                        flush_avz(m, f8s_m)
                        pic, ppr, pjb = decode(m)
                        if pjb == NJB - 1:
                            normalize(pic, ppr)

                # drain the last lagged attn@v steps, then the last
                # chunk's out-projection
                while pend_avs:
                    m, f8s_m = pend_avs.popleft()
                    flush_avz(m, f8s_m)
                    pic, ppr, pjb = decode(m)
                    if pjb == NJB - 1:
                        normalize(pic, ppr)
                for st in range(8):
                    p3_step(IC - 1, st, tail=True)
    nc.finalize()
    return nc


def _shard_inputs(x, w_qkv, b_qkv, w_out):
    """Host-side sharding: per-core input dicts (see module docstring)."""
    x = np.asarray(x, dtype=np.float32)
    w_qkv = np.asarray(w_qkv, dtype=np.float32)
    b_qkv = np.asarray(b_qkv, dtype=np.float32)
    w_out = np.asarray(w_out, dtype=np.float32)

    has_bias = bool(np.any(b_qkv))
    kt = DIM // P + (1 if has_bias else 0)

    # lo/hi head-major reorder for the q and k column blocks (DoubleRow
    # layout): [h0.d0:32 | h1.d0:32 | h2.d0:32 | h3.d0:32 | h0.d32:64 | ...]
    perm = np.concatenate(
        [np.arange(h * DH + 32 * i, h * DH + 32 * i + 32)
         for i in range(2) for h in range(H_LOC)]
    )

    in_maps = []
    for c in range(NCORES):
        b = c // HGROUPS
        hg = c % HGROUPS
        fsl = slice(hg * F_LOC, (hg + 1) * F_LOC)
        w_shard = np.concatenate(
            [
                (w_qkv[:, 0 * DIM:1 * DIM][:, fsl] * SCALE)[:, perm],
                w_qkv[:, 1 * DIM:2 * DIM][:, fsl][:, perm],
                w_qkv[:, 2 * DIM:3 * DIM][:, fsl],
            ],
            axis=1,
        )
        xT_aug = np.zeros((kt * P, N), dtype=np.float32)
        xT_aug[:DIM] = x[b].T
        w_aug = np.zeros((kt * P, 3 * F_LOC), dtype=np.float32)
        w_aug[:DIM] = w_shard
        if has_bias:
            xT_aug[DIM] = 1.0
            w_aug[DIM] = np.concatenate(
                [
                    (b_qkv[0 * DIM:1 * DIM][fsl] * SCALE)[perm],
                    b_qkv[1 * DIM:2 * DIM][fsl][perm],
                    b_qkv[2 * DIM:3 * DIM][fsl],
                ]
            )
        xT_bf = xT_aug.astype(ml_dtypes.bfloat16)
        w_bf = w_aug.astype(ml_dtypes.bfloat16)
        # colsum of the device's v = bf16(x).T-tiles @ bf16(w_v) psum, for
        # the f8 expm1 pullback: csum[0:64, h] = sum_j v[j, 64h+d], rows
        # 64-127 = 2048.0 (the Z offset; sum_j 1 over the key axis)
        v_dev = (
            xT_bf.astype(np.float32).T @
            w_bf[:, 2 * F_LOC:3 * F_LOC].astype(np.float32)
        )
        if has_bias:
            # the ones-row is real x for the projection; colsum over
            # tokens only (rows 0:N are the tokens regardless)
            pass
        csum = np.empty((P, H_LOC + 1), dtype=np.float32)
        csum[:, H_LOC] = float(N)
        csum[64:128, 0:H_LOC] = float(N)
        for h in range(H_LOC):
            csum[0:64, h] = v_dev[:, 64 * h:64 * h + 64].sum(axis=0)
        in_maps.append(
            {
                "xT": np.ascontiguousarray(xT_bf),
                "w": np.ascontiguousarray(w_bf),
                "wo": np.ascontiguousarray(
                    w_out[fsl, :].astype(ml_dtypes.bfloat16)
                ),
                "csum": np.ascontiguousarray(csum),
            }
        )
    return in_maps, kt


def _run(x, w_qkv, b_qkv, b_out, w_out, trace=False, **spmd_kwargs):
    in_maps, kt = _shard_inputs(x, w_qkv, b_qkv, w_out)
    nc = build_nc(kt)
    res = run_bass_kernel_spmd(
        nc, in_maps, core_ids=list(range(NCORES)), trace=trace, **spmd_kwargs
    )
    b_out = np.asarray(b_out, dtype=np.float32)
    full = np.empty((B, N, DIM), dtype=np.float32)
    for b in range(B):
        acc = res.results[b * HGROUPS]["out"].astype(np.float32)
        for hg in range(1, HGROUPS):
            acc = acc + res.results[b * HGROUPS + hg]["out"].astype(np.float32)
        full[b] = acc + b_out
    return full, res


def kernel(x, w_qkv, b_qkv, w_out, b_out):
    full, _ = _run(x, w_qkv, b_qkv, b_out, w_out, trace=False)
    return full
